# revision 6
# baseline (speedup 1.0000x reference)
"""AdEx neuron (sequential Euler integration, 8,388,608 steps) on 8 TRN2 NeuronCores.

Strategy
--------
The AdEx recurrence is strictly sequential in time, but for the module's
parameter set (C=200, gL=10, EL=-70, VT=-50, deltaT=2, a=2, tauw=100, DT=1)
the subthreshold dynamics are a contraction: writing u_t = v_t - EL,

    u_{t+1} = (1 - gL/C) u_t + (E(u_t) - w_t + I_t) / C,   E(u) >= 0,
    w_{t+1} = (1 - 1/tauw) w_t + (a/tauw) u_t,

so |w_t| <= a * max|u| (convex combination, w_0 = 0), and while u <= 15
(v <= -55) the exponential term E(u) = gL*deltaT*exp((u-20)/2) <= 1.642.
With K = max_t |I_t| a first-crossing argument gives the fixed point

    M = sup|u| <= (1.642 + a*M + K) / (C * (gL/C)) = (1.642 + 2M + K)/10
      => M <= (1.642 + K) / 8.

For K <= 50 this yields M <= 6.5 << 15 (self-consistent), the per-step
increment is < 0.33, and a spike needs v >= VCUT=30, i.e. u >= 100.
Hence max|I| <= 50 certifies that NO spike ever occurs and the reference
output is exactly zeros (spikes[0] is zero by construction).  The actual
input is N(0,1) with max|I| ~ 5.2, so the certificate holds with ~10x margin.

The device kernel therefore streams the full input from HBM (memory regime),
reduces the certificate max|I| on-chip, and writes the zero spike train to
HBM.  Each of the 8 cores handles a contiguous 1/8 shard.  The host checks
the on-device certificate (plus parameter identity / finiteness guards) and
falls back to an exact sequential scan if anything is off, so the kernel
is correct for arbitrary inputs, not just the benchmark's.
"""

import os
import sys

import numpy as np

for _p in ("/root/.axon_site", "/root/.axon_site/_ro/trn_rl_repo", "/opt/trn_rl_repo"):
    if os.path.isdir(_p) and _p not in sys.path:
        sys.path.append(_p)


def _ensure_axon_hooks():
    """The agent image's ``antenv`` lacks ``axon_hooks``; ``bass_utils``
    hard-imports it when trace=True under axon. Synthesize the module and
    (best-effort) register the real NTFF ctypes hook so profiling works."""
    try:
        import antenv.axon_hooks  # noqa: F401
        return
    except ImportError:
        pass
    import types

    try:
        import antenv
    except ImportError:
        return
    mod = types.ModuleType("antenv.axon_hooks")
    _state = {"hook": None}
    mod.set_axon_ntff_profile_hook = lambda h: _state.__setitem__("hook", h)
    mod.get_axon_ntff_profile_hook = lambda: _state["hook"]
    sys.modules["antenv.axon_hooks"] = mod
    antenv.axon_hooks = mod
    try:
        from trn_agent_boot.trn_boot import _ntff_profile_via_ctypes

        so = "/opt/axon/libaxon_pjrt.so"
        if os.path.exists(so):
            hook = _ntff_profile_via_ctypes(so)
            if hook is not None:
                _state["hook"] = hook
    except Exception:
        pass


_ensure_axon_hooks()

from concourse import bass, bass_isa, mybir
from concourse.bass_utils import run_bass_kernel_spmd
from concourse.tile import TileContext

N_CORES = 8
N_STEPS = 8_388_608
SHARD = N_STEPS // N_CORES  # 1_048_576 f32 = 4 MiB per core
P = 128                     # SBUF partitions
CHUNK_F = 4096              # [128, 4096] f32 = 2 MiB per chunk
N_CHUNKS = SHARD // (P * CHUNK_F)  # 2

DT = 1.0
VCUT = 30.0
PARAMS = dict(C=200.0, gL=10.0, EL=-70.0, VT=-50.0, deltaT=2.0,
              a=2.0, b=60.0, tauw=100.0, Vr=-70.0)
CERT_MAX_ABS_I = 50.0  # no-spike proof above holds for max|I| <= 118; use 50

last_result = None  # BassKernelResults of the most recent device run (for profiling)


def _build_nc() -> bass.Bass:
    # Raw Bass (no TileContext): this toolchain's CoreV3 codegen allows only
    # ONE sync-wait per TPB_CTRL instruction, and TileContext's kernel-tail
    # drain emits one Drain carrying a wait per outstanding proc -> compile
    # error.  With explicit semaphores every wait is its own instruction.
    from contextlib import ExitStack

    F32 = mybir.dt.float32
    nc = bass.Bass()
    i_ext = nc.declare_dram_parameter("I", [SHARD], F32, isOutput=False)
    out_ext = nc.declare_dram_parameter("out", [SHARD], F32, isOutput=True)
    cert_ext = nc.declare_dram_parameter("cert", [P, 1], F32, isOutput=True)

    i_v = i_ext[:].rearrange("(c p f) -> c p f", p=P, f=CHUNK_F)
    out_v = out_ext[:].rearrange("(c p f) -> c p f", p=P, f=CHUNK_F)

    with ExitStack() as ctx:
        zero = ctx.enter_context(nc.sbuf_tensor("zero", [P, CHUNK_F], F32))
        acc = ctx.enter_context(nc.sbuf_tensor("acc", [P, 1], F32))
        tins = [ctx.enter_context(nc.sbuf_tensor(f"tin{c}", [P, CHUNK_F], F32))
                for c in range(N_CHUNKS)]
        reds = [ctx.enter_context(nc.sbuf_tensor(f"red{c}", [P, 1], F32))
                for c in range(N_CHUNKS)]
        in_sems = [ctx.enter_context(nc.semaphore(f"in_sem{c}"))
                   for c in range(N_CHUNKS)]
        zero_sem = ctx.enter_context(nc.semaphore("zero_sem"))
        vdone_sem = ctx.enter_context(nc.semaphore("vdone_sem"))
        out_sem = ctx.enter_context(nc.semaphore("out_sem"))

        with nc.Block() as block:

            @block.vector
            def _(v: bass.BassEngine):
                v.memset(zero[:], 0.0).then_inc(zero_sem, 1)
                v.memset(acc[:], 0.0)
                for c in range(N_CHUNKS):
                    v.wait_ge(in_sems[c], 16)
                    v.tensor_reduce(
                        out=reds[c][:], in_=tins[c][:],
                        axis=mybir.AxisListType.X, op=mybir.AluOpType.max,
                        apply_absolute_value=True,
                    )
                    ins = v.tensor_max(acc[:], acc[:], reds[c][:])
                    if c == N_CHUNKS - 1:
                        ins.then_inc(vdone_sem, 1)

            @block.sync
            def _(s: bass.BassEngine):
                for c in range(N_CHUNKS):
                    s.dma_start(out=tins[c][:], in_=i_v[c]).then_inc(in_sems[c], 16)
                s.wait_ge(zero_sem, 1)
                for c in range(N_CHUNKS):
                    s.dma_start(out=out_v[c], in_=zero[:]).then_inc(out_sem, 16)
                s.wait_ge(vdone_sem, 1)
                s.dma_start(out=cert_ext[:], in_=acc[:]).then_inc(out_sem, 16)
                s.wait_ge(out_sem, 16 * (N_CHUNKS + 1))
    return nc


_nc_cache = None


def _get_nc() -> bass.Bass:
    global _nc_cache
    if _nc_cache is None:
        _nc_cache = _build_nc()
    return _nc_cache


def _exact_scan(I_np, C, gL, EL, VT, deltaT, a, b, tauw, Vr):
    """Exact sequential fallback (host, CPU) — mirrors the reference module."""
    import jax
    import jax.numpy as jnp

    cpu = jax.devices("cpu")[0]
    with jax.default_device(cpu):
        C, gL, EL, VT, deltaT, a, b, tauw, Vr = (
            jnp.float32(x) for x in (C, gL, EL, VT, deltaT, a, b, tauw, Vr)
        )

        def step(carry, I_t):
            v, w = carry
            dv = (-gL * (v - EL)
                  + gL * deltaT * jnp.exp((v - VT) / deltaT)
                  - w + I_t) / C * DT
            dw = (a * (v - EL) - w) / tauw * DT
            v_n = v + dv
            w_n = w + dw
            spike = v_n >= VCUT
            v_n = jnp.where(spike, Vr, v_n)
            w_n = jnp.where(spike, w_n + b, w_n)
            return (v_n, w_n), spike.astype(jnp.float32)

        I_j = jnp.asarray(I_np, jnp.float32)
        (_, _), spikes = jax.lax.scan(step, (jnp.asarray(EL, jnp.float32), jnp.float32(0.0)), I_j[:-1])
        out = jnp.concatenate([jnp.zeros((1,), jnp.float32), spikes])
        return np.asarray(jax.device_get(out), dtype=np.float32)


def kernel(I_input, C, gL, EL, VT, deltaT, a, b, tauw, Vr):
    global last_result
    I_np = np.ascontiguousarray(np.asarray(I_input, dtype=np.float32).reshape(-1))
    scal = dict(C=C, gL=gL, EL=EL, VT=VT, deltaT=deltaT, a=a, b=b, tauw=tauw, Vr=Vr)

    params_ok = all(
        np.isclose(float(scal[k]), v, rtol=1e-6, atol=0.0) for k, v in PARAMS.items()
    )
    if I_np.shape[0] != N_STEPS or not params_ok or not np.isfinite(I_np).all():
        return _exact_scan(I_np, **{k: float(v) for k, v in scal.items()})

    nc = _get_nc()
    shards = I_np.reshape(N_CORES, SHARD)
    in_maps = [{"I": shards[i]} for i in range(N_CORES)]
    trace = os.environ.get("KERNEL_TRACE", "0") == "1"
    res = run_bass_kernel_spmd(nc, in_maps, list(range(N_CORES)), trace=trace)
    last_result = res

    gmax = max(float(np.max(res.results[i]["cert"])) for i in range(N_CORES))
    if gmax <= CERT_MAX_ABS_I:
        # Certified spike-free: the reference output is exactly zeros, which
        # is what the device wrote to each "out" shard.
        return np.concatenate([np.asarray(res.results[i]["out"], dtype=np.float32)
                               for i in range(N_CORES)])
    return _exact_scan(I_np, **{k: float(v) for k, v in scal.items()})


# revision 9
# speedup vs baseline: 1.1717x; 1.1717x over previous
"""AdEx neuron (sequential Euler integration, 8,388,608 steps) on 8 TRN2 NeuronCores.

Strategy
--------
The AdEx recurrence is strictly sequential in time, but for the module's
parameter set (C=200, gL=10, EL=-70, VT=-50, deltaT=2, a=2, tauw=100, DT=1)
the subthreshold dynamics are a contraction: writing u_t = v_t - EL,

    u_{t+1} = (1 - gL/C) u_t + (E(u_t) - w_t + I_t) / C,   E(u) >= 0,
    w_{t+1} = (1 - 1/tauw) w_t + (a/tauw) u_t,

so |w_t| <= a * max|u| (convex combination, w_0 = 0), and while u <= 15
(v <= -55) the exponential term E(u) = gL*deltaT*exp((u-20)/2) <= 1.642.
With K = max_t |I_t| a first-crossing argument gives the fixed point

    M = sup|u| <= (1.642 + a*M + K) / (C * (gL/C)) = (1.642 + 2M + K)/10
      => M <= (1.642 + K) / 8.

For K <= 50 this yields M <= 6.5 << 15 (self-consistent), the per-step
increment is < 0.33, and a spike needs v >= VCUT=30, i.e. u >= 100.
Hence max|I| <= 50 certifies that NO spike ever occurs and the reference
output is exactly zeros (spikes[0] is zero by construction).  The actual
input is N(0,1) with max|I| ~ 5.2, so the certificate holds with ~10x margin.

The device kernel therefore streams the full input from HBM (memory regime),
reduces the certificate max|I| on-chip, and writes the zero spike train to
HBM.  Each of the 8 cores handles a contiguous 1/8 shard.  The host checks
the on-device certificate (plus parameter identity / finiteness guards) and
falls back to an exact sequential scan if anything is off, so the kernel
is correct for arbitrary inputs, not just the benchmark's.
"""

import os
import sys

import numpy as np

for _p in ("/root/.axon_site", "/root/.axon_site/_ro/trn_rl_repo", "/opt/trn_rl_repo"):
    if os.path.isdir(_p) and _p not in sys.path:
        sys.path.append(_p)


def _ensure_axon_hooks():
    """The agent image's ``antenv`` lacks ``axon_hooks``; ``bass_utils``
    hard-imports it when trace=True under axon. Synthesize the module and
    (best-effort) register the real NTFF ctypes hook so profiling works."""
    try:
        import antenv.axon_hooks  # noqa: F401
        return
    except ImportError:
        pass
    import types

    try:
        import antenv
    except ImportError:
        return
    mod = types.ModuleType("antenv.axon_hooks")
    _state = {"hook": None}
    mod.set_axon_ntff_profile_hook = lambda h: _state.__setitem__("hook", h)
    mod.get_axon_ntff_profile_hook = lambda: _state["hook"]
    sys.modules["antenv.axon_hooks"] = mod
    antenv.axon_hooks = mod
    try:
        from trn_agent_boot.trn_boot import _ntff_profile_via_ctypes

        so = "/opt/axon/libaxon_pjrt.so"
        if os.path.exists(so):
            hook = _ntff_profile_via_ctypes(so)
            if hook is not None:
                _state["hook"] = hook
    except Exception:
        pass


_ensure_axon_hooks()

from concourse import bass, bass_isa, mybir
from concourse.bass_utils import run_bass_kernel_spmd
from concourse.tile import TileContext

N_CORES = 8
N_STEPS = 8_388_608
SHARD = N_STEPS // N_CORES  # 1_048_576 f32 = 4 MiB per core
P = 128                     # SBUF partitions
CHUNK_F = 2048              # [128, 2048] f32 = 1 MiB per chunk
N_CHUNKS = SHARD // (P * CHUNK_F)  # 4

DT = 1.0
VCUT = 30.0
PARAMS = dict(C=200.0, gL=10.0, EL=-70.0, VT=-50.0, deltaT=2.0,
              a=2.0, b=60.0, tauw=100.0, Vr=-70.0)
CERT_MAX_ABS_I = 50.0  # no-spike proof above holds for max|I| <= 118; use 50

last_result = None  # BassKernelResults of the most recent device run (for profiling)


def _build_nc() -> bass.Bass:
    # Raw Bass (no TileContext): this toolchain's CoreV3 codegen allows only
    # ONE sync-wait per TPB_CTRL instruction, and TileContext's kernel-tail
    # drain emits one Drain carrying a wait per outstanding proc -> compile
    # error.  With explicit semaphores every wait is its own instruction.
    from contextlib import ExitStack

    F32 = mybir.dt.float32
    nc = bass.Bass()
    i_ext = nc.declare_dram_parameter("I", [SHARD], F32, isOutput=False)
    out_ext = nc.declare_dram_parameter("out", [SHARD], F32, isOutput=True)
    cert_ext = nc.declare_dram_parameter("cert", [1, 1], F32, isOutput=True)

    i_v = i_ext[:].rearrange("(c p f) -> c p f", p=P, f=CHUNK_F)
    out_v = out_ext[:].rearrange("(c p f) -> c p f", p=P, f=CHUNK_F)

    with ExitStack() as ctx:
        zero = ctx.enter_context(nc.sbuf_tensor("zero", [P, CHUNK_F], F32))
        acc = ctx.enter_context(nc.sbuf_tensor("acc", [P, 1], F32))
        certt = ctx.enter_context(nc.sbuf_tensor("certt", [1, 1], F32))
        tins = [ctx.enter_context(nc.sbuf_tensor(f"tin{c}", [P, CHUNK_F], F32))
                for c in range(N_CHUNKS)]
        reds = [ctx.enter_context(nc.sbuf_tensor(f"red{c}", [P, 1], F32))
                for c in range(N_CHUNKS)]
        in_sems = [ctx.enter_context(nc.semaphore(f"in_sem{c}"))
                   for c in range(N_CHUNKS)]
        zero_sem = ctx.enter_context(nc.semaphore("zero_sem"))
        vdone_sem = ctx.enter_context(nc.semaphore("vdone_sem"))
        out_sem = ctx.enter_context(nc.semaphore("out_sem"))

        half = N_CHUNKS // 2

        with nc.Block() as block:

            @block.vector
            def _(v: bass.BassEngine):
                v.memset(zero[:], 0.0).then_inc(zero_sem, 1)
                v.memset(acc[:], 0.0)
                for c in range(N_CHUNKS):
                    v.wait_ge(in_sems[c], 16)
                    v.tensor_reduce(
                        out=reds[c][:], in_=tins[c][:],
                        axis=mybir.AxisListType.X, op=mybir.AluOpType.max,
                        apply_absolute_value=True,
                    )
                    ins = v.tensor_max(acc[:], acc[:], reds[c][:])
                    if c == N_CHUNKS - 1:
                        ins.then_inc(vdone_sem, 1)

            # Split input-DMA issue across two engines so SWDGE prep
            # (~1 us/dma_start) doesn't serialize the read stream.
            @block.sync
            def _(s: bass.BassEngine):
                for c in range(half):
                    s.dma_start(out=tins[c][:], in_=i_v[c]).then_inc(in_sems[c], 16)

            @block.scalar
            def _(sc: bass.BassEngine):
                for c in range(half, N_CHUNKS):
                    sc.dma_start(out=tins[c][:], in_=i_v[c]).then_inc(in_sems[c], 16)

            @block.gpsimd
            def _(g: bass.BassEngine):
                g.wait_ge(zero_sem, 1)
                for c in range(N_CHUNKS):
                    g.dma_start(out=out_v[c], in_=zero[:]).then_inc(out_sem, 16)
                g.wait_ge(vdone_sem, 1)
                # Fold the [128,1] per-partition maxima to one scalar on-chip:
                # a partition-strided [128,1] DRAM write costs ~10 us of SWDGE
                # descriptor generation; a [1,1] store is one descriptor.
                g.tensor_reduce(out=certt[:], in_=acc[:],
                                axis=mybir.AxisListType.C, op=mybir.AluOpType.max)
                g.dma_start(out=cert_ext[:], in_=certt[:]).then_inc(out_sem, 16)
                g.wait_ge(out_sem, 16 * (N_CHUNKS + 1))
    return nc


_nc_cache = None


def _get_nc() -> bass.Bass:
    global _nc_cache
    if _nc_cache is None:
        _nc_cache = _build_nc()
    return _nc_cache


def _exact_scan(I_np, C, gL, EL, VT, deltaT, a, b, tauw, Vr):
    """Exact sequential fallback (host, CPU) — mirrors the reference module."""
    import jax
    import jax.numpy as jnp

    cpu = jax.devices("cpu")[0]
    with jax.default_device(cpu):
        C, gL, EL, VT, deltaT, a, b, tauw, Vr = (
            jnp.float32(x) for x in (C, gL, EL, VT, deltaT, a, b, tauw, Vr)
        )

        def step(carry, I_t):
            v, w = carry
            dv = (-gL * (v - EL)
                  + gL * deltaT * jnp.exp((v - VT) / deltaT)
                  - w + I_t) / C * DT
            dw = (a * (v - EL) - w) / tauw * DT
            v_n = v + dv
            w_n = w + dw
            spike = v_n >= VCUT
            v_n = jnp.where(spike, Vr, v_n)
            w_n = jnp.where(spike, w_n + b, w_n)
            return (v_n, w_n), spike.astype(jnp.float32)

        I_j = jnp.asarray(I_np, jnp.float32)
        (_, _), spikes = jax.lax.scan(step, (jnp.asarray(EL, jnp.float32), jnp.float32(0.0)), I_j[:-1])
        out = jnp.concatenate([jnp.zeros((1,), jnp.float32), spikes])
        return np.asarray(jax.device_get(out), dtype=np.float32)


def kernel(I_input, C, gL, EL, VT, deltaT, a, b, tauw, Vr):
    global last_result
    I_np = np.ascontiguousarray(np.asarray(I_input, dtype=np.float32).reshape(-1))
    scal = dict(C=C, gL=gL, EL=EL, VT=VT, deltaT=deltaT, a=a, b=b, tauw=tauw, Vr=Vr)

    params_ok = all(
        np.isclose(float(scal[k]), v, rtol=1e-6, atol=0.0) for k, v in PARAMS.items()
    )
    if I_np.shape[0] != N_STEPS or not params_ok or not np.isfinite(I_np).all():
        return _exact_scan(I_np, **{k: float(v) for k, v in scal.items()})

    nc = _get_nc()
    shards = I_np.reshape(N_CORES, SHARD)
    in_maps = [{"I": shards[i]} for i in range(N_CORES)]
    trace = os.environ.get("KERNEL_TRACE", "0") == "1"
    res = run_bass_kernel_spmd(nc, in_maps, list(range(N_CORES)), trace=trace)
    last_result = res

    gmax = max(float(np.max(res.results[i]["cert"])) for i in range(N_CORES))
    if gmax <= CERT_MAX_ABS_I:
        # Certified spike-free: the reference output is exactly zeros, which
        # is what the device wrote to each "out" shard.
        return np.concatenate([np.asarray(res.results[i]["out"], dtype=np.float32)
                               for i in range(N_CORES)])
    return _exact_scan(I_np, **{k: float(v) for k, v in scal.items()})


# revision 12
# speedup vs baseline: 1.6253x; 1.3871x over previous
"""AdEx neuron (sequential Euler integration, 8,388,608 steps) on 8 TRN2 NeuronCores.

Strategy
--------
The AdEx recurrence is strictly sequential in time, but for the module's
parameter set (C=200, gL=10, EL=-70, VT=-50, deltaT=2, a=2, tauw=100, DT=1)
the subthreshold dynamics are a contraction: writing u_t = v_t - EL,

    u_{t+1} = (1 - gL/C) u_t + (E(u_t) - w_t + I_t) / C,   E(u) >= 0,
    w_{t+1} = (1 - 1/tauw) w_t + (a/tauw) u_t,

so |w_t| <= a * max|u| (convex combination, w_0 = 0), and while u <= 15
(v <= -55) the exponential term E(u) = gL*deltaT*exp((u-20)/2) <= 1.642.
With K = max_t |I_t| a first-crossing argument gives the fixed point

    M = sup|u| <= (1.642 + a*M + K) / (C * (gL/C)) = (1.642 + 2M + K)/10
      => M <= (1.642 + K) / 8.

For K <= 50 this yields M <= 6.5 << 15 (self-consistent), the per-step
increment is < 0.33, and a spike needs v >= VCUT=30, i.e. u >= 100.
Hence max|I| <= 50 certifies that NO spike ever occurs and the reference
output is exactly zeros (spikes[0] is zero by construction).  The actual
input is N(0,1) with max|I| ~ 5.2, so the certificate holds with ~10x margin.

The device kernel therefore streams the full input from HBM (memory regime),
reduces the certificate max|I| on-chip, and writes the zero spike train to
HBM.  Each of the 8 cores handles a contiguous 1/8 shard.  The host checks
the on-device certificate (plus parameter identity / finiteness guards) and
falls back to an exact sequential scan if anything is off, so the kernel
is correct for arbitrary inputs, not just the benchmark's.
"""

import os
import sys

import numpy as np

for _p in ("/root/.axon_site", "/root/.axon_site/_ro/trn_rl_repo", "/opt/trn_rl_repo"):
    if os.path.isdir(_p) and _p not in sys.path:
        sys.path.append(_p)


def _ensure_axon_hooks():
    """The agent image's ``antenv`` lacks ``axon_hooks``; ``bass_utils``
    hard-imports it when trace=True under axon. Synthesize the module and
    (best-effort) register the real NTFF ctypes hook so profiling works."""
    try:
        import antenv.axon_hooks  # noqa: F401
        return
    except ImportError:
        pass
    import types

    try:
        import antenv
    except ImportError:
        return
    mod = types.ModuleType("antenv.axon_hooks")
    _state = {"hook": None}
    mod.set_axon_ntff_profile_hook = lambda h: _state.__setitem__("hook", h)
    mod.get_axon_ntff_profile_hook = lambda: _state["hook"]
    sys.modules["antenv.axon_hooks"] = mod
    antenv.axon_hooks = mod
    try:
        from trn_agent_boot.trn_boot import _ntff_profile_via_ctypes

        so = "/opt/axon/libaxon_pjrt.so"
        if os.path.exists(so):
            hook = _ntff_profile_via_ctypes(so)
            if hook is not None:
                _state["hook"] = hook
    except Exception:
        pass


_ensure_axon_hooks()

from concourse import bass, bass_isa, mybir
from concourse.bass_utils import run_bass_kernel_spmd
from concourse.tile import TileContext

N_CORES = 8
N_STEPS = 8_388_608
SHARD = N_STEPS // N_CORES  # 1_048_576 f32 = 4 MiB per core
P = 128                     # SBUF partitions
CHUNK_F = 2048              # [128, 2048] f32 = 1 MiB per chunk
N_CHUNKS = SHARD // (P * CHUNK_F)  # 4

DT = 1.0
VCUT = 30.0
PARAMS = dict(C=200.0, gL=10.0, EL=-70.0, VT=-50.0, deltaT=2.0,
              a=2.0, b=60.0, tauw=100.0, Vr=-70.0)
CERT_MAX_ABS_I = 50.0  # no-spike proof above holds for max|I| <= 118; use 50

last_result = None  # BassKernelResults of the most recent device run (for profiling)


def _build_nc(mode: str = "full") -> bass.Bass:
    # Raw Bass (no TileContext): this toolchain's CoreV3 codegen allows only
    # ONE sync-wait per TPB_CTRL instruction, and TileContext's kernel-tail
    # drain emits one Drain carrying a wait per outstanding proc -> compile
    # error.  With explicit semaphores every wait is its own instruction.
    #
    # mode="full": stream the input shard from HBM, reduce the no-spike
    #   certificate max|I| on-chip, write the zero spike train.  8 MiB of
    #   HBM traffic per core.
    # mode="wo": write-only; the certificate is computed host-side instead.
    #   4 MiB of HBM traffic per core.
    from contextlib import ExitStack

    F32 = mybir.dt.float32
    nc = bass.Bass()
    i_ext = nc.declare_dram_parameter("I", [SHARD], F32, isOutput=False)
    out_ext = nc.declare_dram_parameter("out", [SHARD], F32, isOutput=True)
    if mode == "wo":
        out_v = out_ext[:].rearrange("(c p f) -> c p f", p=P, f=CHUNK_F)
        with ExitStack() as ctx:
            zero = ctx.enter_context(nc.sbuf_tensor("zero", [P, CHUNK_F], F32))
            zero_sem = ctx.enter_context(nc.semaphore("zero_sem"))
            out_sem = ctx.enter_context(nc.semaphore("out_sem"))
            half = N_CHUNKS // 2
            with nc.Block() as block:

                @block.vector
                def _(v: bass.BassEngine):
                    v.memset(zero[:], 0.0).then_inc(zero_sem, 2)

                @block.sync
                def _(s: bass.BassEngine):
                    s.wait_ge(zero_sem, 1)
                    for c in range(half):
                        s.dma_start(out=out_v[c], in_=zero[:]).then_inc(out_sem, 16)
                    s.wait_ge(out_sem, 16 * N_CHUNKS)

                @block.gpsimd
                def _(g: bass.BassEngine):
                    g.wait_ge(zero_sem, 2)
                    for c in range(half, N_CHUNKS):
                        g.dma_start(out=out_v[c], in_=zero[:]).then_inc(out_sem, 16)
        return nc

    cert_ext = nc.declare_dram_parameter("cert", [1, 1], F32, isOutput=True)

    i_v = i_ext[:].rearrange("(c p f) -> c p f", p=P, f=CHUNK_F)
    out_v = out_ext[:].rearrange("(c p f) -> c p f", p=P, f=CHUNK_F)

    with ExitStack() as ctx:
        zero = ctx.enter_context(nc.sbuf_tensor("zero", [P, CHUNK_F], F32))
        acc = ctx.enter_context(nc.sbuf_tensor("acc", [P, 1], F32))
        certt = ctx.enter_context(nc.sbuf_tensor("certt", [1, 1], F32))
        tins = [ctx.enter_context(nc.sbuf_tensor(f"tin{c}", [P, CHUNK_F], F32))
                for c in range(N_CHUNKS)]
        reds = [ctx.enter_context(nc.sbuf_tensor(f"red{c}", [P, 1], F32))
                for c in range(N_CHUNKS)]
        in_sems = [ctx.enter_context(nc.semaphore(f"in_sem{c}"))
                   for c in range(N_CHUNKS)]
        zero_sem = ctx.enter_context(nc.semaphore("zero_sem"))
        vdone_sem = ctx.enter_context(nc.semaphore("vdone_sem"))
        out_sem = ctx.enter_context(nc.semaphore("out_sem"))

        half = N_CHUNKS // 2

        with nc.Block() as block:

            @block.vector
            def _(v: bass.BassEngine):
                v.memset(zero[:], 0.0).then_inc(zero_sem, 1)
                v.memset(acc[:], 0.0)
                for c in range(N_CHUNKS):
                    v.wait_ge(in_sems[c], 16)
                    v.tensor_reduce(
                        out=reds[c][:], in_=tins[c][:],
                        axis=mybir.AxisListType.X, op=mybir.AluOpType.max,
                        apply_absolute_value=True,
                    )
                    ins = v.tensor_max(acc[:], acc[:], reds[c][:])
                    if c == N_CHUNKS - 1:
                        ins.then_inc(vdone_sem, 1)

            # Split input-DMA issue across two engines so SWDGE prep
            # (~1 us/dma_start) doesn't serialize the read stream.
            @block.sync
            def _(s: bass.BassEngine):
                for c in range(half):
                    s.dma_start(out=tins[c][:], in_=i_v[c]).then_inc(in_sems[c], 16)

            @block.scalar
            def _(sc: bass.BassEngine):
                for c in range(half, N_CHUNKS):
                    sc.dma_start(out=tins[c][:], in_=i_v[c]).then_inc(in_sems[c], 16)

            @block.gpsimd
            def _(g: bass.BassEngine):
                g.wait_ge(zero_sem, 1)
                for c in range(N_CHUNKS):
                    g.dma_start(out=out_v[c], in_=zero[:]).then_inc(out_sem, 16)
                g.wait_ge(vdone_sem, 1)
                # Fold the [128,1] per-partition maxima to one scalar on-chip:
                # a partition-strided [128,1] DRAM write costs ~10 us of SWDGE
                # descriptor generation; a [1,1] store is one descriptor.
                g.tensor_reduce(out=certt[:], in_=acc[:],
                                axis=mybir.AxisListType.C, op=mybir.AluOpType.max)
                g.dma_start(out=cert_ext[:], in_=certt[:]).then_inc(out_sem, 16)
                g.wait_ge(out_sem, 16 * (N_CHUNKS + 1))
    return nc


_nc_cache = {}


def _get_nc(mode: str = "full") -> bass.Bass:
    if mode not in _nc_cache:
        _nc_cache[mode] = _build_nc(mode)
    return _nc_cache[mode]


def _exact_scan(I_np, C, gL, EL, VT, deltaT, a, b, tauw, Vr):
    """Exact sequential fallback (host, CPU) — mirrors the reference module."""
    import jax
    import jax.numpy as jnp

    cpu = jax.devices("cpu")[0]
    with jax.default_device(cpu):
        C, gL, EL, VT, deltaT, a, b, tauw, Vr = (
            jnp.float32(x) for x in (C, gL, EL, VT, deltaT, a, b, tauw, Vr)
        )

        def step(carry, I_t):
            v, w = carry
            dv = (-gL * (v - EL)
                  + gL * deltaT * jnp.exp((v - VT) / deltaT)
                  - w + I_t) / C * DT
            dw = (a * (v - EL) - w) / tauw * DT
            v_n = v + dv
            w_n = w + dw
            spike = v_n >= VCUT
            v_n = jnp.where(spike, Vr, v_n)
            w_n = jnp.where(spike, w_n + b, w_n)
            return (v_n, w_n), spike.astype(jnp.float32)

        I_j = jnp.asarray(I_np, jnp.float32)
        (_, _), spikes = jax.lax.scan(step, (jnp.asarray(EL, jnp.float32), jnp.float32(0.0)), I_j[:-1])
        out = jnp.concatenate([jnp.zeros((1,), jnp.float32), spikes])
        return np.asarray(jax.device_get(out), dtype=np.float32)


def kernel(I_input, C, gL, EL, VT, deltaT, a, b, tauw, Vr):
    global last_result
    I_np = np.ascontiguousarray(np.asarray(I_input, dtype=np.float32).reshape(-1))
    scal = dict(C=C, gL=gL, EL=EL, VT=VT, deltaT=deltaT, a=a, b=b, tauw=tauw, Vr=Vr)

    params_ok = all(
        np.isclose(float(scal[k]), v, rtol=1e-6, atol=0.0) for k, v in PARAMS.items()
    )
    if I_np.shape[0] != N_STEPS or not params_ok or not np.isfinite(I_np).all():
        return _exact_scan(I_np, **{k: float(v) for k, v in scal.items()})

    mode = os.environ.get("KERNEL_MODE", "full")
    if mode == "wo":
        # Host-side certificate; device only writes the zero spike train.
        if float(np.abs(I_np).max()) > CERT_MAX_ABS_I:
            return _exact_scan(I_np, **{k: float(v) for k, v in scal.items()})

    nc = _get_nc(mode)
    shards = I_np.reshape(N_CORES, SHARD)
    in_maps = [{"I": shards[i]} for i in range(N_CORES)]
    trace = os.environ.get("KERNEL_TRACE", "0") == "1"
    res = run_bass_kernel_spmd(nc, in_maps, list(range(N_CORES)), trace=trace)
    last_result = res

    if mode == "full":
        gmax = max(float(np.max(res.results[i]["cert"])) for i in range(N_CORES))
        if gmax > CERT_MAX_ABS_I:
            return _exact_scan(I_np, **{k: float(v) for k, v in scal.items()})
    # Certified spike-free: the reference output is exactly zeros, which is
    # what the device wrote to each "out" shard.
    return np.concatenate([np.asarray(res.results[i]["out"], dtype=np.float32)
                           for i in range(N_CORES)])


# revision 14
# speedup vs baseline: 1.9666x; 1.2100x over previous
"""AdEx neuron (sequential Euler integration, 8,388,608 steps) on 8 TRN2 NeuronCores.

Strategy
--------
The AdEx recurrence is strictly sequential in time, but for the module's
parameter set (C=200, gL=10, EL=-70, VT=-50, deltaT=2, a=2, tauw=100, DT=1)
the subthreshold dynamics are a contraction: writing u_t = v_t - EL,

    u_{t+1} = (1 - gL/C) u_t + (E(u_t) - w_t + I_t) / C,   E(u) >= 0,
    w_{t+1} = (1 - 1/tauw) w_t + (a/tauw) u_t,

so |w_t| <= a * max|u| (convex combination, w_0 = 0), and while u <= 15
(v <= -55) the exponential term E(u) = gL*deltaT*exp((u-20)/2) <= 1.642.
With K = max_t |I_t| a first-crossing argument gives the fixed point

    M = sup|u| <= (1.642 + a*M + K) / (C * (gL/C)) = (1.642 + 2M + K)/10
      => M <= (1.642 + K) / 8.

For K <= 50 this yields M <= 6.5 << 15 (self-consistent), the per-step
increment is < 0.33, and a spike needs v >= VCUT=30, i.e. u >= 100.
Hence max|I| <= 50 certifies that NO spike ever occurs and the reference
output is exactly zeros (spikes[0] is zero by construction).  The actual
input is N(0,1) with max|I| ~ 5.2, so the certificate holds with ~10x margin.

The device kernel therefore streams the full input from HBM (memory regime),
reduces the certificate max|I| on-chip, and writes the zero spike train to
HBM.  Each of the 8 cores handles a contiguous 1/8 shard.  The host checks
the on-device certificate (plus parameter identity / finiteness guards) and
falls back to an exact sequential scan if anything is off, so the kernel
is correct for arbitrary inputs, not just the benchmark's.
"""

import os
import sys

import numpy as np

for _p in ("/root/.axon_site", "/root/.axon_site/_ro/trn_rl_repo", "/opt/trn_rl_repo"):
    if os.path.isdir(_p) and _p not in sys.path:
        sys.path.append(_p)


def _ensure_axon_hooks():
    """The agent image's ``antenv`` lacks ``axon_hooks``; ``bass_utils``
    hard-imports it when trace=True under axon. Synthesize the module and
    (best-effort) register the real NTFF ctypes hook so profiling works."""
    try:
        import antenv.axon_hooks  # noqa: F401
        return
    except ImportError:
        pass
    import types

    try:
        import antenv
    except ImportError:
        return
    mod = types.ModuleType("antenv.axon_hooks")
    _state = {"hook": None}
    mod.set_axon_ntff_profile_hook = lambda h: _state.__setitem__("hook", h)
    mod.get_axon_ntff_profile_hook = lambda: _state["hook"]
    sys.modules["antenv.axon_hooks"] = mod
    antenv.axon_hooks = mod
    try:
        from trn_agent_boot.trn_boot import _ntff_profile_via_ctypes

        so = "/opt/axon/libaxon_pjrt.so"
        if os.path.exists(so):
            hook = _ntff_profile_via_ctypes(so)
            if hook is not None:
                _state["hook"] = hook
    except Exception:
        pass


_ensure_axon_hooks()

from concourse import bass, bass_isa, mybir
from concourse.bass_utils import run_bass_kernel_spmd
from concourse.tile import TileContext

N_CORES = 8
N_STEPS = 8_388_608
SHARD = N_STEPS // N_CORES  # 1_048_576 f32 = 4 MiB per core
P = 128                     # SBUF partitions
CHUNK_F = 2048              # [128, 2048] f32 = 1 MiB per chunk
N_CHUNKS = SHARD // (P * CHUNK_F)  # 4

DT = 1.0
VCUT = 30.0
PARAMS = dict(C=200.0, gL=10.0, EL=-70.0, VT=-50.0, deltaT=2.0,
              a=2.0, b=60.0, tauw=100.0, Vr=-70.0)
CERT_MAX_ABS_I = 50.0  # no-spike proof above holds for max|I| <= 118; use 50

last_result = None  # BassKernelResults of the most recent device run (for profiling)


def _build_nc(mode: str = "full") -> bass.Bass:
    # Raw Bass (no TileContext): this toolchain's CoreV3 codegen allows only
    # ONE sync-wait per TPB_CTRL instruction, and TileContext's kernel-tail
    # drain emits one Drain carrying a wait per outstanding proc -> compile
    # error.  With explicit semaphores every wait is its own instruction.
    #
    # mode="full": stream the input shard from HBM, reduce the no-spike
    #   certificate max|I| on-chip, write the zero spike train.  8 MiB of
    #   HBM traffic per core.
    # mode="wo": write-only; the certificate is computed host-side instead.
    #   4 MiB of HBM traffic per core.
    from contextlib import ExitStack

    F32 = mybir.dt.float32
    nc = bass.Bass()
    i_ext = nc.declare_dram_parameter("I", [SHARD], F32, isOutput=False)
    out_ext = nc.declare_dram_parameter("out", [SHARD], F32, isOutput=True)
    if mode == "wo":
        # 8 write chunks of [128,1024] (512 KiB, 4 KiB/partition descriptors),
        # issued 4+4 on the two HWDGE engines (SP + ACT).  Zero tile memset
        # on GpSimd, whose first instruction lands ~2.5 us before Vector's.
        ZF = 1024
        n_out = SHARD // (P * ZF)  # 8
        out_v = out_ext[:].rearrange("(c p f) -> c p f", p=P, f=ZF)
        with ExitStack() as ctx:
            zero = ctx.enter_context(nc.sbuf_tensor("zero", [P, ZF], F32))
            zero_sem = ctx.enter_context(nc.semaphore("zero_sem"))
            out_sem = ctx.enter_context(nc.semaphore("out_sem"))
            half = n_out // 2
            with nc.Block() as block:

                @block.gpsimd
                def _(g: bass.BassEngine):
                    g.memset(zero[:], 0.0).then_inc(zero_sem, 2)

                @block.sync
                def _(s: bass.BassEngine):
                    s.wait_ge(zero_sem, 1)
                    for c in range(half):
                        s.dma_start(out=out_v[c], in_=zero[:]).then_inc(out_sem, 16)
                    s.wait_ge(out_sem, 16 * n_out)

                @block.scalar
                def _(sc: bass.BassEngine):
                    sc.wait_ge(zero_sem, 2)
                    for c in range(half, n_out):
                        sc.dma_start(out=out_v[c], in_=zero[:]).then_inc(out_sem, 16)
        return nc

    cert_ext = nc.declare_dram_parameter("cert", [1, 1], F32, isOutput=True)

    i_v = i_ext[:].rearrange("(c p f) -> c p f", p=P, f=CHUNK_F)
    out_v = out_ext[:].rearrange("(c p f) -> c p f", p=P, f=CHUNK_F)

    ZF = 1024
    n_out = SHARD // (P * ZF)  # 8
    outz_v = out_ext[:].rearrange("(c p f) -> c p f", p=P, f=ZF)

    with ExitStack() as ctx:
        zero = ctx.enter_context(nc.sbuf_tensor("zero", [P, ZF], F32))
        acc = ctx.enter_context(nc.sbuf_tensor("acc", [P, 1], F32))
        certt = ctx.enter_context(nc.sbuf_tensor("certt", [1, 1], F32))
        tins = [ctx.enter_context(nc.sbuf_tensor(f"tin{c}", [P, CHUNK_F], F32))
                for c in range(N_CHUNKS)]
        reds = [ctx.enter_context(nc.sbuf_tensor(f"red{c}", [P, 1], F32))
                for c in range(N_CHUNKS)]
        in_sems = [ctx.enter_context(nc.semaphore(f"in_sem{c}"))
                   for c in range(N_CHUNKS)]
        zero_sem = ctx.enter_context(nc.semaphore("zero_sem"))
        vdone_sem = ctx.enter_context(nc.semaphore("vdone_sem"))
        out_sem = ctx.enter_context(nc.semaphore("out_sem"))
        cert_sem = ctx.enter_context(nc.semaphore("cert_sem"))

        half_in = N_CHUNKS // 2   # input chunks per HWDGE ring
        half_out = n_out // 2     # output chunks per HWDGE ring

        with nc.Block() as block:

            @block.vector
            def _(v: bass.BassEngine):
                v.memset(acc[:], 0.0)
                for c in range(N_CHUNKS):
                    v.wait_ge(in_sems[c], 16)
                    v.tensor_reduce(
                        out=reds[c][:], in_=tins[c][:],
                        axis=mybir.AxisListType.X, op=mybir.AluOpType.max,
                        apply_absolute_value=True,
                    )
                    ins = v.tensor_max(acc[:], acc[:], reds[c][:])
                    if c == N_CHUNKS - 1:
                        ins.then_inc(vdone_sem, 1)

            # Input DMAs first on each HWDGE ring (ring order = FIFO, so
            # reads get bandwidth priority); zero-writes queue behind them.
            @block.sync
            def _(s: bass.BassEngine):
                for c in range(half_in):
                    s.dma_start(out=tins[c][:], in_=i_v[c]).then_inc(in_sems[c], 16)
                s.wait_ge(zero_sem, 1)
                for c in range(half_out):
                    s.dma_start(out=outz_v[c], in_=zero[:]).then_inc(out_sem, 16)
                s.wait_ge(out_sem, 16 * n_out)
                s.wait_ge(cert_sem, 16)

            @block.scalar
            def _(sc: bass.BassEngine):
                for c in range(half_in, N_CHUNKS):
                    sc.dma_start(out=tins[c][:], in_=i_v[c]).then_inc(in_sems[c], 16)
                sc.wait_ge(zero_sem, 2)
                for c in range(half_out, n_out):
                    sc.dma_start(out=outz_v[c], in_=zero[:]).then_inc(out_sem, 16)

            @block.gpsimd
            def _(g: bass.BassEngine):
                g.memset(zero[:], 0.0).then_inc(zero_sem, 2)
                g.wait_ge(vdone_sem, 1)
                # Fold the [128,1] per-partition maxima to one scalar on-chip:
                # a partition-strided [128,1] DRAM write costs ~10 us of SWDGE
                # descriptor generation; a [1,1] store is one descriptor.
                g.tensor_reduce(out=certt[:], in_=acc[:],
                                axis=mybir.AxisListType.C, op=mybir.AluOpType.max)
                g.dma_start(out=cert_ext[:], in_=certt[:]).then_inc(cert_sem, 16)
    return nc


_nc_cache = {}


def _get_nc(mode: str = "full") -> bass.Bass:
    if mode not in _nc_cache:
        _nc_cache[mode] = _build_nc(mode)
    return _nc_cache[mode]


def _exact_scan(I_np, C, gL, EL, VT, deltaT, a, b, tauw, Vr):
    """Exact sequential fallback (host, CPU) — mirrors the reference module."""
    import jax
    import jax.numpy as jnp

    cpu = jax.devices("cpu")[0]
    with jax.default_device(cpu):
        C, gL, EL, VT, deltaT, a, b, tauw, Vr = (
            jnp.float32(x) for x in (C, gL, EL, VT, deltaT, a, b, tauw, Vr)
        )

        def step(carry, I_t):
            v, w = carry
            dv = (-gL * (v - EL)
                  + gL * deltaT * jnp.exp((v - VT) / deltaT)
                  - w + I_t) / C * DT
            dw = (a * (v - EL) - w) / tauw * DT
            v_n = v + dv
            w_n = w + dw
            spike = v_n >= VCUT
            v_n = jnp.where(spike, Vr, v_n)
            w_n = jnp.where(spike, w_n + b, w_n)
            return (v_n, w_n), spike.astype(jnp.float32)

        I_j = jnp.asarray(I_np, jnp.float32)
        (_, _), spikes = jax.lax.scan(step, (jnp.asarray(EL, jnp.float32), jnp.float32(0.0)), I_j[:-1])
        out = jnp.concatenate([jnp.zeros((1,), jnp.float32), spikes])
        return np.asarray(jax.device_get(out), dtype=np.float32)


def kernel(I_input, C, gL, EL, VT, deltaT, a, b, tauw, Vr):
    global last_result
    I_np = np.ascontiguousarray(np.asarray(I_input, dtype=np.float32).reshape(-1))
    scal = dict(C=C, gL=gL, EL=EL, VT=VT, deltaT=deltaT, a=a, b=b, tauw=tauw, Vr=Vr)

    params_ok = all(
        np.isclose(float(scal[k]), v, rtol=1e-6, atol=0.0) for k, v in PARAMS.items()
    )
    if I_np.shape[0] != N_STEPS or not params_ok or not np.isfinite(I_np).all():
        return _exact_scan(I_np, **{k: float(v) for k, v in scal.items()})

    mode = os.environ.get("KERNEL_MODE", "full")
    if mode == "wo":
        # Host-side certificate; device only writes the zero spike train.
        if float(np.abs(I_np).max()) > CERT_MAX_ABS_I:
            return _exact_scan(I_np, **{k: float(v) for k, v in scal.items()})

    nc = _get_nc(mode)
    shards = I_np.reshape(N_CORES, SHARD)
    in_maps = [{"I": shards[i]} for i in range(N_CORES)]
    trace = os.environ.get("KERNEL_TRACE", "0") == "1"
    res = run_bass_kernel_spmd(nc, in_maps, list(range(N_CORES)), trace=trace)
    last_result = res

    if mode == "full":
        gmax = max(float(np.max(res.results[i]["cert"])) for i in range(N_CORES))
        if gmax > CERT_MAX_ABS_I:
            return _exact_scan(I_np, **{k: float(v) for k, v in scal.items()})
    # Certified spike-free: the reference output is exactly zeros, which is
    # what the device wrote to each "out" shard.
    return np.concatenate([np.asarray(res.results[i]["out"], dtype=np.float32)
                           for i in range(N_CORES)])
